# revision 25
# baseline (speedup 1.0000x reference)
"""Trainium2 Bass kernel: BlockAttnRes forward (v2).

Reference computation (per batch b, position t):
    k[n]   = s[n] / sqrt(mean(s[n]^2) + eps)        n in [0, 9)
    score  = k[n] . w                                (w = queries[layer_idx])
    alpha  = softmax(score over n)
    h[t]   = sum_n alpha[n] * s[n]                   (d = 512)

Distribution: batch dim B=8 -> one batch per NeuronCore, no cross-core
communication.  Per core: T=4096 positions in 16 MACRO-tiles of 2x128
(two partition-tiles j=0,1 per macro -> small ops amortize over both).

Engine budget per macro (2 tiles, ~16us each on DVE/ACT = the wall):
    DMA   : 6 chunk loads [128,3,512] f32 (6KB rows) + 2 stores bf16
    ACT   : 18x Square+accum (ssq), Ln+Exp (rsq), 1x Exp (e), 1 PSUM copy
    DVE   : 18x dot STT+accum, score mul, seg max/sum, max-subtract via
            stride-0 broadcast add, recip, 2x diag STT (I*rs_j*e_j
            broadcast), 1 PSUM copy (bf16 cast)
    PE    : 18 accumulating fp32r matmuls (bitcast; 1 cyc/row vs 4 for f32)
Emission is software-pipelined across 3 macros so each engine's in-order
queue always holds ready work ahead of cross-engine waits. GpSimd is left
idle on purpose: it shares a DVE SBUF port and measurably slows the dot
STTs (tested: diag on GpSimd = +35us wall).

All ACT funcs (Square/Ln/Exp/Copy) in `natural_log_exp_and_others` ->
one ACT_TABLE_LOAD (pinned via PinnedBacc; stock chooser thrashes sets).
"""

import numpy as np

B, T, N, D = 8, 4096, 9, 512
P = 128
EPS = 1e-6
NCORES = 8
JT = 2              # partition-tiles per macro iteration
MACRO = P * JT      # 256 positions per macro

_CACHE = {}


def _build_bass(
    t_len=T,
    diag_engine="vector",   # "gpsimd" | "vector"
    n_act_sq=18,            # squares on ACT (rest on DVE via STT)
    out_bf16=True,
):
    import concourse.bass as bass
    import concourse.tile as tile
    from concourse import bacc, mybir

    f32 = mybir.dt.float32
    f32r = mybir.dt.float32r
    bf16 = mybir.dt.bfloat16
    Alu = mybir.AluOpType
    Act = mybir.ActivationFunctionType
    Ax = mybir.AxisListType

    nmacro = t_len // MACRO
    out_dt = bf16 if out_bf16 else f32

    PINNED_SET = "natural_log_exp_and_others"

    class PinnedBacc(bacc.Bacc):
        def insert_act_table_loads(self):
            import bass_rust as _bass_rust
            from concourse.hw_specs import get_activation_tables

            all_tables = get_activation_tables(self.m.arch)
            used = {
                i.func
                for b in self.main_func.blocks
                for i in b.instructions
                if isinstance(i, mybir.InstActivation)
            }
            if used and PINNED_SET in all_tables and used <= all_tables[PINNED_SET]:
                tables = [
                    (name, funcs if name == PINNED_SET else set())
                    for name, funcs in all_tables.items()
                ]
            else:
                tables = list(all_tables.items())
            _bass_rust.insert_act_table_loads(self, tables)

    nc = PinnedBacc("TRN2", target_bir_lowering=False, debug=False)
    src = nc.dram_tensor("src", [t_len, N, D], f32, kind="ExternalInput").ap()
    wq = nc.dram_tensor("wq", [P, D], f32, kind="ExternalInput").ap()
    idn = nc.dram_tensor("idn", [P, P], f32, kind="ExternalInput").ap()
    out = nc.dram_tensor("out", [t_len, D], out_dt, kind="ExternalOutput").ap()

    src_t = src.rearrange("(c j p) n d -> c j p n d", j=JT, p=P)
    out_t = out.rearrange("(c j p) d -> c j p d", j=JT, p=P)

    NCH = 3           # n-chunks per partition-tile
    CN = N // NCH     # n's per chunk

    def bc(ap, reps):
        """Insert a stride-0 dim after the partition dim."""
        return bass.AP(
            tensor=ap.tensor,
            offset=ap.offset,
            ap=[ap.ap[0], [0, reps], *ap.ap[1:]],
        )

    def bc_inner(ap, reps):
        """Append a stride-0 innermost dim."""
        return bass.AP(
            tensor=ap.tensor,
            offset=ap.offset,
            ap=[*ap.ap, [0, reps]],
        )

    with tile.TileContext(nc) as tc:
        with (
            tc.tile_pool(name="const", bufs=1) as const_pool,
            tc.tile_pool(name="srcp", bufs=4) as src_pool,
            tc.tile_pool(name="scratch", bufs=3) as scr_pool,
            tc.tile_pool(name="small", bufs=4) as small_pool,
            tc.tile_pool(name="diag", bufs=3) as diag_pool,
            tc.tile_pool(name="hout", bufs=4) as out_pool,
            tc.tile_pool(name="psum", bufs=4, space="PSUM") as psum_pool,
        ):
            state = {}
            const_tiles = {}

            def emit_w():
                # w gates the first dot STTs -> its DMA goes first of all.
                w_sb = const_pool.tile([P, D], f32, name="w_sb")
                nc.sync.dma_start(out=w_sb, in_=wq)
                eps_sb = const_pool.tile([P, 1], f32, name="eps_sb")
                nc.vector.memset(eps_sb, EPS)
                const_tiles.update(w=w_sb, eps=eps_sb)

            def emit_idn():
                # identity is first needed by diag in emit_back(0), much later
                i_sb = const_pool.tile([P, P], f32, name="i_sb")
                nc.sync.dma_start(out=i_sb, in_=idn)
                const_tiles.update(i=i_sb)

            def emit_loads(c):
                # Tiles are float32r-typed (verifier demands fp32r matmul
                # inputs come from fp32r locations); DVE/ACT consumers read
                # them bitcast back to f32 — same bytes.
                chunks = [[None] * NCH for _ in range(JT)]
                for j in range(JT):
                    for k in range(NCH):
                        sk = src_pool.tile([P, CN, D], f32r, tag=f"s{j}{k}")
                        nc.sync.dma_start(
                            out=sk,
                            in_=src_t[c, j, :, k * CN : (k + 1) * CN, :].bitcast(f32r),
                        )
                        chunks[j][k] = sk
                state[c] = {"chunks": chunks}

            def s_mm(c, j, n):
                return state[c]["chunks"][j][n // CN][:, n % CN, :]

            def s_of(c, j, n):
                return s_mm(c, j, n).bitcast(f32)

            def emit_passes(c):
                """Bulk streaming passes: ssq (ACT), dots (DVE), rsq (ACT)."""
                st = state[c]
                ssq = small_pool.tile([P, JT, N], f32, tag="ssq")
                sq = scr_pool.tile([P, D], f32, tag="sq")
                for j in range(JT):
                    for n in range(N):
                        nc.scalar.activation(
                            out=sq,
                            in_=s_of(c, j, n),
                            func=Act.Square,
                            accum_out=ssq[:, j, n : n + 1],
                        )
                dot = small_pool.tile([P, JT, N], f32, tag="dot")
                prod = scr_pool.tile([P, D], f32, tag="prod")
                for j in range(JT):
                    for n in range(N):
                        nc.vector.scalar_tensor_tensor(
                            out=prod,
                            in0=s_of(c, j, n),
                            scalar=0.0,
                            in1=const_tiles["w"],
                            op0=Alu.bypass,
                            op1=Alu.mult,
                            accum_out=dot[:, j, n : n + 1],
                        )
                # rsq = (ssq/D + eps)^(-1/2) via Exp(-0.5*Ln(x))
                rsq = small_pool.tile([P, JT, N], f32, tag="rsq")
                nc.scalar.activation(
                    out=rsq,
                    in_=ssq,
                    func=Act.Ln,
                    scale=1.0 / D,
                    bias=const_tiles["eps"],
                )
                nc.scalar.activation(out=rsq, in_=rsq, func=Act.Exp, scale=-0.5)
                st["dot"], st["rsq"] = dot, rsq

            def emit_front(c):
                """score + (negated) row max on DVE."""
                st = state[c]
                score = small_pool.tile([P, JT, N], f32, tag="score")
                nc.vector.tensor_mul(score, st["dot"], st["rsq"])
                nmx = small_pool.tile([P, JT], f32, tag="nmx")
                nc.vector.tensor_reduce(
                    out=nmx, in_=score, axis=Ax.X, op=Alu.max, negate=True
                )
                st["score"], st["nmx"] = score, nmx

            def emit_exp(c):
                """e = exp(score - max): subtract the per-j max on DVE via
                a stride-0 broadcast add (nmx is stored negated), then ONE
                ACT Exp over both tiles. Emitted late so the DVE inputs are
                long done when ACT reaches the Exp."""
                st = state[c]
                score2 = small_pool.tile([P, JT, N], f32, tag="score2")
                nc.vector.tensor_add(score2, st["score"], bc_inner(st["nmx"], N))
                e = small_pool.tile([P, JT, N], f32, tag="e")
                nc.scalar.activation(out=e, in_=score2, func=Act.Exp)
                st["e"] = e

            def emit_back(c):
                """sume/recip/diag on DVE + the 18 fp32r matmuls."""
                st = state[c]
                e = st["e"]
                sume = small_pool.tile([P, JT], f32, tag="sume")
                nc.vector.tensor_reduce(out=sume, in_=e, axis=Ax.X, op=Alu.add)
                rs = small_pool.tile([P, JT], f32, tag="rs")
                nc.vector.reciprocal(out=rs, in_=sume)

                # diag(alpha): either one GpSimd tensor mul over a
                # pre-built alpha (Pool has no scalar-AP ops), or per-j
                # DVE STTs folding rs — interleaved with that j's matmuls.
                dg = diag_pool.tile([P, JT * N, P], f32r, tag="dg")
                if diag_engine == "gpsimd":
                    al = small_pool.tile([P, JT * N], f32, tag="al")
                    for j in range(JT):
                        nc.vector.tensor_scalar_mul(
                            al[:, j * N : (j + 1) * N], e[:, j, :], rs[:, j : j + 1]
                        )
                    nc.gpsimd.tensor_tensor(
                        out=dg,
                        in0=bc(const_tiles["i"], JT * N),
                        in1=bc_inner(al, P),
                        op=Alu.mult,
                    )
                hps = []
                for j in range(JT):
                    if diag_engine != "gpsimd":
                        nc.vector.scalar_tensor_tensor(
                            out=dg[:, j * N : (j + 1) * N, :],
                            in0=bc(const_tiles["i"], N),
                            scalar=rs[:, j : j + 1],
                            in1=bc_inner(e[:, j, :], P),
                            op0=Alu.mult,
                            op1=Alu.mult,
                        )
                    hp = psum_pool.tile([P, D], f32, tag=f"hp{j}")
                    for n in range(N):
                        nc.tensor.matmul(
                            hp,
                            dg[:, j * N + n, :],
                            s_mm(c, j, n),
                            start=(n == 0),
                            stop=(n == N - 1),
                        )
                    hps.append(hp)
                st["hps"] = hps

            def emit_copies(c):
                """PSUM -> SBUF (+ bf16 cast) + stores; j=0 ACT, j=1 DVE."""
                st = state[c]
                hs = out_pool.tile([P, JT, D], out_dt, tag="hs")
                for j in range(JT):
                    hp = st["hps"][j]
                    if j == 0 and diag_engine != "gpsimd":
                        nc.scalar.activation(out=hs[:, j, :], in_=hp, func=Act.Copy)
                    else:
                        nc.vector.tensor_copy(out=hs[:, j, :], in_=hp)
                    nc.sync.dma_start(out=out_t[c, j], in_=hs[:, j, :])
                del state[c]

            # Software-pipelined emission. Per iteration i the engine queues
            # see (in order):
            #   DVE: sume/recip/diag(i-1), cast(i-2)... score/nmx(i), dots(i+1)
            #   ACT: squares(i+1)+Ln/Exp(i+1), exp(i), copy(i-1)
            #   PE : matmuls(i-1)
            # so every cross-engine wait lands behind a long runway of
            # already-ready work.
            emit_w()
            emit_loads(0)
            emit_loads(1)
            emit_idn()
            emit_passes(0)
            for c in range(nmacro):
                if c >= 1:
                    emit_back(c - 1)
                emit_front(c)
                if c + 2 < nmacro:
                    emit_loads(c + 2)
                if c + 1 < nmacro:
                    emit_passes(c + 1)
                emit_exp(c)
                if c >= 1:
                    emit_copies(c - 1)
            emit_back(nmacro - 1)
            emit_copies(nmacro - 1)

    nc.compile()
    return nc


def _get_nc(t_len=T, **kw):
    key = (t_len, tuple(sorted(kw.items())))
    if key not in _CACHE:
        _CACHE[key] = _build_bass(t_len, **kw)
    return _CACHE[key]


def _make_in_maps(sources, queries, layer_idx):
    sources = np.ascontiguousarray(np.asarray(sources, dtype=np.float32))
    queries = np.asarray(queries, dtype=np.float32)
    w = queries[int(layer_idx)]
    w_rep = np.ascontiguousarray(np.broadcast_to(w[None, :], (P, D)).astype(np.float32))
    idn = np.eye(P, dtype=np.float32)
    return [
        {"src": np.ascontiguousarray(sources[b]), "wq": w_rep, "idn": idn}
        for b in range(sources.shape[0])
    ]


def kernel(sources, queries, layer_idx):
    from concourse.bass_utils import run_bass_kernel_spmd

    nc = _get_nc()
    in_maps = _make_in_maps(sources, queries, layer_idx)
    res = run_bass_kernel_spmd(nc, in_maps, core_ids=list(range(NCORES)))
    outs = [
        np.asarray(res.results[b]["out"]).astype(np.float32) for b in range(NCORES)
    ]
    return np.stack(outs, axis=0)


# revision 26
# speedup vs baseline: 1.0096x; 1.0096x over previous
"""Trainium2 Bass kernel: BlockAttnRes forward (v2).

Reference computation (per batch b, position t):
    k[n]   = s[n] / sqrt(mean(s[n]^2) + eps)        n in [0, 9)
    score  = k[n] . w                                (w = queries[layer_idx])
    alpha  = softmax(score over n)
    h[t]   = sum_n alpha[n] * s[n]                   (d = 512)

Distribution: batch dim B=8 -> one batch per NeuronCore, no cross-core
communication.  Per core: T=4096 positions in 16 MACRO-tiles of 2x128
(two partition-tiles j=0,1 per macro -> small ops amortize over both).

Engine budget per macro (2 tiles, ~16us each on DVE/ACT = the wall):
    DMA   : 6 chunk loads [128,3,512] f32 (6KB rows) + 2 stores bf16
    ACT   : 18x Square+accum (ssq), Ln+Exp (rsq), 1x Exp (e), 1 PSUM copy
    DVE   : 18x dot STT+accum, score mul, seg max/sum, max-subtract via
            stride-0 broadcast add, recip, 2x diag STT (I*rs_j*e_j
            broadcast), 1 PSUM copy (bf16 cast)
    PE    : 18 accumulating fp32r matmuls (bitcast; 1 cyc/row vs 4 for f32)
Emission is software-pipelined across 3 macros so each engine's in-order
queue always holds ready work ahead of cross-engine waits. GpSimd is left
idle on purpose: it shares a DVE SBUF port and measurably slows the dot
STTs (tested: diag on GpSimd = +35us wall).

All ACT funcs (Square/Ln/Exp/Copy) in `natural_log_exp_and_others` ->
one ACT_TABLE_LOAD (pinned via PinnedBacc; stock chooser thrashes sets).
"""

import numpy as np

B, T, N, D = 8, 4096, 9, 512
P = 128
EPS = 1e-6
NCORES = 8
JT = 2              # partition-tiles per macro iteration
MACRO = P * JT      # 256 positions per macro

_CACHE = {}


def _build_bass(
    t_len=T,
    diag_engine="vector",   # "gpsimd" | "vector"
    n_act_sq=18,            # squares on ACT (rest on DVE via STT)
    out_bf16=True,
):
    import concourse.bass as bass
    import concourse.tile as tile
    from concourse import bacc, mybir

    f32 = mybir.dt.float32
    f32r = mybir.dt.float32r
    bf16 = mybir.dt.bfloat16
    Alu = mybir.AluOpType
    Act = mybir.ActivationFunctionType
    Ax = mybir.AxisListType

    nmacro = t_len // MACRO
    out_dt = bf16 if out_bf16 else f32

    PINNED_SET = "natural_log_exp_and_others"

    class PinnedBacc(bacc.Bacc):
        def insert_act_table_loads(self):
            import bass_rust as _bass_rust
            from concourse.hw_specs import get_activation_tables

            all_tables = get_activation_tables(self.m.arch)
            used = {
                i.func
                for b in self.main_func.blocks
                for i in b.instructions
                if isinstance(i, mybir.InstActivation)
            }
            if used and PINNED_SET in all_tables and used <= all_tables[PINNED_SET]:
                tables = [
                    (name, funcs if name == PINNED_SET else set())
                    for name, funcs in all_tables.items()
                ]
            else:
                tables = list(all_tables.items())
            _bass_rust.insert_act_table_loads(self, tables)

    nc = PinnedBacc("TRN2", target_bir_lowering=False, debug=False)
    src = nc.dram_tensor("src", [t_len, N, D], f32, kind="ExternalInput").ap()
    wq = nc.dram_tensor("wq", [P, D], f32, kind="ExternalInput").ap()
    idn = nc.dram_tensor("idn", [P, P], f32, kind="ExternalInput").ap()
    out = nc.dram_tensor("out", [t_len, D], out_dt, kind="ExternalOutput").ap()

    src_t = src.rearrange("(c j p) n d -> c j p n d", j=JT, p=P)
    out_t = out.rearrange("(c j p) d -> c j p d", j=JT, p=P)

    NCH = 3           # n-chunks per partition-tile
    CN = N // NCH     # n's per chunk

    def bc(ap, reps):
        """Insert a stride-0 dim after the partition dim."""
        return bass.AP(
            tensor=ap.tensor,
            offset=ap.offset,
            ap=[ap.ap[0], [0, reps], *ap.ap[1:]],
        )

    def bc_inner(ap, reps):
        """Append a stride-0 innermost dim."""
        return bass.AP(
            tensor=ap.tensor,
            offset=ap.offset,
            ap=[*ap.ap, [0, reps]],
        )

    with tile.TileContext(nc) as tc:
        with (
            tc.tile_pool(name="const", bufs=1) as const_pool,
            tc.tile_pool(name="srcp", bufs=4) as src_pool,
            tc.tile_pool(name="scratch", bufs=3) as scr_pool,
            tc.tile_pool(name="small", bufs=4) as small_pool,
            tc.tile_pool(name="diag", bufs=3) as diag_pool,
            tc.tile_pool(name="hout", bufs=4) as out_pool,
            tc.tile_pool(name="psum", bufs=4, space="PSUM") as psum_pool,
        ):
            state = {}
            const_tiles = {}

            def emit_w():
                # w gates the first dot STTs -> its DMA goes first of all.
                w_sb = const_pool.tile([P, D], f32, name="w_sb")
                nc.sync.dma_start(out=w_sb, in_=wq)
                eps_sb = const_pool.tile([P, 1], f32, name="eps_sb")
                nc.vector.memset(eps_sb, EPS)
                const_tiles.update(w=w_sb, eps=eps_sb)

            def emit_idn():
                # identity is first needed by diag in emit_back(0), much later
                i_sb = const_pool.tile([P, P], f32, name="i_sb")
                nc.sync.dma_start(out=i_sb, in_=idn)
                const_tiles.update(i=i_sb)

            def emit_loads(c):
                # Tiles are float32r-typed (verifier demands fp32r matmul
                # inputs come from fp32r locations); DVE/ACT consumers read
                # them bitcast back to f32 — same bytes.
                chunks = [[None] * NCH for _ in range(JT)]
                for j in range(JT):
                    for k in range(NCH):
                        sk = src_pool.tile([P, CN, D], f32r, tag=f"s{j}{k}")
                        nc.sync.dma_start(
                            out=sk,
                            in_=src_t[c, j, :, k * CN : (k + 1) * CN, :].bitcast(f32r),
                        )
                        chunks[j][k] = sk
                state[c] = {"chunks": chunks}

            def s_mm(c, j, n):
                return state[c]["chunks"][j][n // CN][:, n % CN, :]

            def s_of(c, j, n):
                return s_mm(c, j, n).bitcast(f32)

            def emit_passes(c):
                """Bulk streaming passes: ssq (ACT), dots (DVE), rsq (ACT)."""
                st = state[c]
                ssq = small_pool.tile([P, JT, N], f32, tag="ssq")
                sq = scr_pool.tile([P, D], f32, tag="sq")
                for j in range(JT):
                    for n in range(N):
                        nc.scalar.activation(
                            out=sq,
                            in_=s_of(c, j, n),
                            func=Act.Square,
                            accum_out=ssq[:, j, n : n + 1],
                        )
                dot = small_pool.tile([P, JT, N], f32, tag="dot")
                prod = scr_pool.tile([P, D], f32, tag="prod")
                for j in range(JT):
                    for n in range(N):
                        nc.vector.scalar_tensor_tensor(
                            out=prod,
                            in0=s_of(c, j, n),
                            scalar=0.0,
                            in1=const_tiles["w"],
                            op0=Alu.bypass,
                            op1=Alu.mult,
                            accum_out=dot[:, j, n : n + 1],
                        )
                # rsq = (ssq/D + eps)^(-1/2) via Exp(-0.5*Ln(x))
                rsq = small_pool.tile([P, JT, N], f32, tag="rsq")
                nc.scalar.activation(
                    out=rsq,
                    in_=ssq,
                    func=Act.Ln,
                    scale=1.0 / D,
                    bias=const_tiles["eps"],
                )
                nc.scalar.activation(out=rsq, in_=rsq, func=Act.Exp, scale=-0.5)
                st["dot"], st["rsq"] = dot, rsq

            def emit_front(c):
                """score + (negated) row max on DVE."""
                st = state[c]
                score = small_pool.tile([P, JT, N], f32, tag="score")
                nc.vector.tensor_mul(score, st["dot"], st["rsq"])
                nmx = small_pool.tile([P, JT], f32, tag="nmx")
                nc.vector.tensor_reduce(
                    out=nmx, in_=score, axis=Ax.X, op=Alu.max, negate=True
                )
                st["score"], st["nmx"] = score, nmx

            def emit_exp(c):
                """e = exp(score - max): subtract the per-j max on DVE via
                a stride-0 broadcast add (nmx is stored negated), then ONE
                ACT Exp over both tiles. Emitted late so the DVE inputs are
                long done when ACT reaches the Exp."""
                st = state[c]
                score2 = small_pool.tile([P, JT, N], f32, tag="score2")
                nc.vector.tensor_add(score2, st["score"], bc_inner(st["nmx"], N))
                e = small_pool.tile([P, JT, N], f32, tag="e")
                nc.scalar.activation(out=e, in_=score2, func=Act.Exp)
                st["e"] = e

            def emit_back(c):
                """sume/recip/diag on DVE + the 18 fp32r matmuls."""
                st = state[c]
                e = st["e"]
                sume = small_pool.tile([P, JT], f32, tag="sume")
                nc.vector.tensor_reduce(out=sume, in_=e, axis=Ax.X, op=Alu.add)
                rs = small_pool.tile([P, JT], f32, tag="rs")
                nc.vector.reciprocal(out=rs, in_=sume)

                # diag(alpha): either one GpSimd tensor mul over a
                # pre-built alpha (Pool has no scalar-AP ops), or per-j
                # DVE STTs folding rs — interleaved with that j's matmuls.
                dg = diag_pool.tile([P, JT * N, P], f32r, tag="dg")
                if diag_engine == "gpsimd":
                    al = small_pool.tile([P, JT * N], f32, tag="al")
                    for j in range(JT):
                        nc.vector.tensor_scalar_mul(
                            al[:, j * N : (j + 1) * N], e[:, j, :], rs[:, j : j + 1]
                        )
                    nc.gpsimd.tensor_tensor(
                        out=dg,
                        in0=bc(const_tiles["i"], JT * N),
                        in1=bc_inner(al, P),
                        op=Alu.mult,
                    )
                hps = []
                for j in range(JT):
                    if diag_engine != "gpsimd":
                        nc.vector.scalar_tensor_tensor(
                            out=dg[:, j * N : (j + 1) * N, :],
                            in0=bc(const_tiles["i"], N),
                            scalar=rs[:, j : j + 1],
                            in1=bc_inner(e[:, j, :], P),
                            op0=Alu.mult,
                            op1=Alu.mult,
                        )
                    hp = psum_pool.tile([P, D], f32, tag=f"hp{j}")
                    for n in range(N):
                        nc.tensor.matmul(
                            hp,
                            dg[:, j * N + n, :],
                            s_mm(c, j, n),
                            start=(n == 0),
                            stop=(n == N - 1),
                        )
                    hps.append(hp)
                st["hps"] = hps

            def emit_copies(c):
                """PSUM -> SBUF (+ bf16 cast) + stores; j=0 ACT, j=1 DVE."""
                st = state[c]
                hs = out_pool.tile([P, JT, D], out_dt, tag="hs")
                for j in range(JT):
                    hp = st["hps"][j]
                    if j == 0 and diag_engine != "gpsimd":
                        nc.scalar.activation(out=hs[:, j, :], in_=hp, func=Act.Copy)
                    else:
                        nc.vector.tensor_copy(out=hs[:, j, :], in_=hp)
                    # SWDGE store from the idle GpSimd queue: a store that
                    # waits on its PSUM copy must not block next macros'
                    # loads in the (in-order) Sync HWDGE queue.
                    nc.gpsimd.dma_start(out=out_t[c, j], in_=hs[:, j, :])
                del state[c]

            # Software-pipelined emission. Per iteration i the engine queues
            # see (in order):
            #   DVE: sume/recip/diag(i-1), cast(i-2)... score/nmx(i), dots(i+1)
            #   ACT: squares(i+1)+Ln/Exp(i+1), exp(i), copy(i-1)
            #   PE : matmuls(i-1)
            # so every cross-engine wait lands behind a long runway of
            # already-ready work.
            emit_w()
            emit_loads(0)
            emit_loads(1)
            emit_idn()
            emit_passes(0)
            for c in range(nmacro):
                if c >= 1:
                    emit_back(c - 1)
                emit_front(c)
                if c + 2 < nmacro:
                    emit_loads(c + 2)
                if c + 1 < nmacro:
                    emit_passes(c + 1)
                emit_exp(c)
                if c >= 1:
                    emit_copies(c - 1)
            emit_back(nmacro - 1)
            emit_copies(nmacro - 1)

    nc.compile()
    return nc


def _get_nc(t_len=T, **kw):
    key = (t_len, tuple(sorted(kw.items())))
    if key not in _CACHE:
        _CACHE[key] = _build_bass(t_len, **kw)
    return _CACHE[key]


def _make_in_maps(sources, queries, layer_idx):
    sources = np.ascontiguousarray(np.asarray(sources, dtype=np.float32))
    queries = np.asarray(queries, dtype=np.float32)
    w = queries[int(layer_idx)]
    w_rep = np.ascontiguousarray(np.broadcast_to(w[None, :], (P, D)).astype(np.float32))
    idn = np.eye(P, dtype=np.float32)
    return [
        {"src": np.ascontiguousarray(sources[b]), "wq": w_rep, "idn": idn}
        for b in range(sources.shape[0])
    ]


def kernel(sources, queries, layer_idx):
    from concourse.bass_utils import run_bass_kernel_spmd

    nc = _get_nc()
    in_maps = _make_in_maps(sources, queries, layer_idx)
    res = run_bass_kernel_spmd(nc, in_maps, core_ids=list(range(NCORES)))
    outs = [
        np.asarray(res.results[b]["out"]).astype(np.float32) for b in range(NCORES)
    ]
    return np.stack(outs, axis=0)


# revision 27
# speedup vs baseline: 1.0194x; 1.0097x over previous
"""Trainium2 Bass kernel: BlockAttnRes forward (v2).

Reference computation (per batch b, position t):
    k[n]   = s[n] / sqrt(mean(s[n]^2) + eps)        n in [0, 9)
    score  = k[n] . w                                (w = queries[layer_idx])
    alpha  = softmax(score over n)
    h[t]   = sum_n alpha[n] * s[n]                   (d = 512)

Distribution: batch dim B=8 -> one batch per NeuronCore, no cross-core
communication.  Per core: T=4096 positions in 16 MACRO-tiles of 2x128
(two partition-tiles j=0,1 per macro -> small ops amortize over both).

Engine budget per macro (2 tiles, ~16us each on DVE/ACT = the wall):
    DMA   : 6 chunk loads [128,3,512] f32 (6KB rows) + 2 stores bf16
    ACT   : 18x Square+accum (ssq), Ln+Exp (rsq), 1x Exp (e), 1 PSUM copy
    DVE   : 18x dot STT+accum, score mul, seg max/sum, max-subtract via
            stride-0 broadcast add, recip, 2x diag STT (I*rs_j*e_j
            broadcast), 1 PSUM copy (bf16 cast)
    PE    : 18 accumulating fp32r matmuls (bitcast; 1 cyc/row vs 4 for f32)
Emission is software-pipelined across 3 macros so each engine's in-order
queue always holds ready work ahead of cross-engine waits. GpSimd is left
idle on purpose: it shares a DVE SBUF port and measurably slows the dot
STTs (tested: diag on GpSimd = +35us wall).

All ACT funcs (Square/Ln/Exp/Copy) in `natural_log_exp_and_others` ->
one ACT_TABLE_LOAD (pinned via PinnedBacc; stock chooser thrashes sets).
"""

import numpy as np

B, T, N, D = 8, 4096, 9, 512
P = 128
EPS = 1e-6
NCORES = 8
JT = 2              # partition-tiles per macro iteration
MACRO = P * JT      # 256 positions per macro

_CACHE = {}


def _build_bass(
    t_len=T,
    diag_engine="vector",   # "gpsimd" | "vector"
    n_act_sq=18,            # squares on ACT (rest on DVE via STT)
    out_bf16=True,
):
    import concourse.bass as bass
    import concourse.tile as tile
    from concourse import bacc, mybir

    f32 = mybir.dt.float32
    f32r = mybir.dt.float32r
    bf16 = mybir.dt.bfloat16
    Alu = mybir.AluOpType
    Act = mybir.ActivationFunctionType
    Ax = mybir.AxisListType

    nmacro = t_len // MACRO
    out_dt = bf16 if out_bf16 else f32

    PINNED_SET = "natural_log_exp_and_others"

    class PinnedBacc(bacc.Bacc):
        def insert_act_table_loads(self):
            import bass_rust as _bass_rust
            from concourse.hw_specs import get_activation_tables

            all_tables = get_activation_tables(self.m.arch)
            used = {
                i.func
                for b in self.main_func.blocks
                for i in b.instructions
                if isinstance(i, mybir.InstActivation)
            }
            if used and PINNED_SET in all_tables and used <= all_tables[PINNED_SET]:
                tables = [
                    (name, funcs if name == PINNED_SET else set())
                    for name, funcs in all_tables.items()
                ]
            else:
                tables = list(all_tables.items())
            _bass_rust.insert_act_table_loads(self, tables)

    nc = PinnedBacc("TRN2", target_bir_lowering=False, debug=False)
    src = nc.dram_tensor("src", [t_len, N, D], f32, kind="ExternalInput").ap()
    wq = nc.dram_tensor("wq", [P, D], f32, kind="ExternalInput").ap()
    idn = nc.dram_tensor("idn", [P, P], f32, kind="ExternalInput").ap()
    out = nc.dram_tensor("out", [t_len, D], out_dt, kind="ExternalOutput").ap()

    src_t = src.rearrange("(c j p) n d -> c j p n d", j=JT, p=P)
    out_t = out.rearrange("(c j p) d -> c j p d", j=JT, p=P)

    NCH = 3           # n-chunks per partition-tile
    CN = N // NCH     # n's per chunk

    def bc(ap, reps):
        """Insert a stride-0 dim after the partition dim."""
        return bass.AP(
            tensor=ap.tensor,
            offset=ap.offset,
            ap=[ap.ap[0], [0, reps], *ap.ap[1:]],
        )

    def bc_inner(ap, reps):
        """Append a stride-0 innermost dim."""
        return bass.AP(
            tensor=ap.tensor,
            offset=ap.offset,
            ap=[*ap.ap, [0, reps]],
        )

    with tile.TileContext(nc) as tc:
        with (
            tc.tile_pool(name="const", bufs=1) as const_pool,
            tc.tile_pool(name="srcp", bufs=4) as src_pool,
            tc.tile_pool(name="scratch", bufs=3) as scr_pool,
            tc.tile_pool(name="small", bufs=4) as small_pool,
            tc.tile_pool(name="diag", bufs=3) as diag_pool,
            tc.tile_pool(name="hout", bufs=4) as out_pool,
            tc.tile_pool(name="psum", bufs=4, space="PSUM") as psum_pool,
        ):
            state = {}
            const_tiles = {}

            def emit_w():
                # w gates the first dot STTs -> its DMA goes first of all.
                w_sb = const_pool.tile([P, D], f32, name="w_sb")
                nc.sync.dma_start(out=w_sb, in_=wq)
                eps_sb = const_pool.tile([P, 1], f32, name="eps_sb")
                nc.vector.memset(eps_sb, EPS)
                const_tiles.update(w=w_sb, eps=eps_sb)

            def emit_idn():
                # identity is first needed by diag in emit_back(0), much later
                i_sb = const_pool.tile([P, P], f32, name="i_sb")
                nc.sync.dma_start(out=i_sb, in_=idn)
                const_tiles.update(i=i_sb)

            def emit_loads(c):
                # Tiles are float32r-typed (verifier demands fp32r matmul
                # inputs come from fp32r locations); DVE/ACT consumers read
                # them bitcast back to f32 — same bytes.
                chunks = [[None] * NCH for _ in range(JT)]
                for j in range(JT):
                    for k in range(NCH):
                        sk = src_pool.tile([P, CN, D], f32r, tag=f"s{j}{k}")
                        nc.sync.dma_start(
                            out=sk,
                            in_=src_t[c, j, :, k * CN : (k + 1) * CN, :].bitcast(f32r),
                        )
                        chunks[j][k] = sk
                state[c] = {"chunks": chunks}

            def s_mm(c, j, n):
                return state[c]["chunks"][j][n // CN][:, n % CN, :]

            def s_of(c, j, n):
                return s_mm(c, j, n).bitcast(f32)

            def emit_passes(c):
                """Bulk streaming passes: ssq (ACT), dots (DVE), rsq (ACT)."""
                st = state[c]
                ssq = small_pool.tile([P, JT, N], f32, tag="ssq")
                sq = scr_pool.tile([P, D], f32, tag="sq")
                for j in range(JT):
                    for n in range(N):
                        nc.scalar.activation(
                            out=sq,
                            in_=s_of(c, j, n),
                            func=Act.Square,
                            accum_out=ssq[:, j, n : n + 1],
                        )
                dot = small_pool.tile([P, JT, N], f32, tag="dot")
                prod = scr_pool.tile([P, D], f32, tag="prod")
                for j in range(JT):
                    for n in range(N):
                        nc.vector.scalar_tensor_tensor(
                            out=prod,
                            in0=s_of(c, j, n),
                            scalar=0.0,
                            in1=const_tiles["w"],
                            op0=Alu.bypass,
                            op1=Alu.mult,
                            accum_out=dot[:, j, n : n + 1],
                        )
                # rsq = (ssq/D + eps)^(-1/2) via Exp(-0.5*Ln(x))
                rsq = small_pool.tile([P, JT, N], f32, tag="rsq")
                nc.scalar.activation(
                    out=rsq,
                    in_=ssq,
                    func=Act.Ln,
                    scale=1.0 / D,
                    bias=const_tiles["eps"],
                )
                nc.scalar.activation(out=rsq, in_=rsq, func=Act.Exp, scale=-0.5)
                st["dot"], st["rsq"] = dot, rsq

            def emit_front(c):
                """score + (negated) row max on DVE."""
                st = state[c]
                score = small_pool.tile([P, JT, N], f32, tag="score")
                nc.vector.tensor_mul(score, st["dot"], st["rsq"])
                nmx = small_pool.tile([P, JT], f32, tag="nmx")
                nc.vector.tensor_reduce(
                    out=nmx, in_=score, axis=Ax.X, op=Alu.max, negate=True
                )
                st["score"], st["nmx"] = score, nmx

            def emit_exp(c):
                """e = exp(score - max): subtract the per-j max on DVE via
                a stride-0 broadcast add (nmx is stored negated), then ONE
                ACT Exp over both tiles. Emitted late so the DVE inputs are
                long done when ACT reaches the Exp."""
                st = state[c]
                score2 = small_pool.tile([P, JT, N], f32, tag="score2")
                nc.vector.tensor_add(score2, st["score"], bc_inner(st["nmx"], N))
                e = small_pool.tile([P, JT, N], f32, tag="e")
                nc.scalar.activation(out=e, in_=score2, func=Act.Exp)
                st["e"] = e

            def emit_back(c):
                """sume/recip/diag on DVE + the 18 fp32r matmuls."""
                st = state[c]
                e = st["e"]
                sume = small_pool.tile([P, JT], f32, tag="sume")
                nc.vector.tensor_reduce(out=sume, in_=e, axis=Ax.X, op=Alu.add)
                rs = small_pool.tile([P, JT], f32, tag="rs")
                nc.vector.reciprocal(out=rs, in_=sume)

                # diag(alpha): either one GpSimd tensor mul over a
                # pre-built alpha (Pool has no scalar-AP ops), or per-j
                # DVE STTs folding rs — interleaved with that j's matmuls.
                dg = diag_pool.tile([P, JT * N, P], f32r, tag="dg")
                if diag_engine == "gpsimd":
                    al = small_pool.tile([P, JT * N], f32, tag="al")
                    for j in range(JT):
                        nc.vector.tensor_scalar_mul(
                            al[:, j * N : (j + 1) * N], e[:, j, :], rs[:, j : j + 1]
                        )
                    nc.gpsimd.tensor_tensor(
                        out=dg,
                        in0=bc(const_tiles["i"], JT * N),
                        in1=bc_inner(al, P),
                        op=Alu.mult,
                    )
                hps = []
                for j in range(JT):
                    if diag_engine != "gpsimd":
                        nc.vector.scalar_tensor_tensor(
                            out=dg[:, j * N : (j + 1) * N, :],
                            in0=bc(const_tiles["i"], N),
                            scalar=rs[:, j : j + 1],
                            in1=bc_inner(e[:, j, :], P),
                            op0=Alu.mult,
                            op1=Alu.mult,
                        )
                    hp = psum_pool.tile([P, D], f32, tag=f"hp{j}")
                    for n in range(N):
                        nc.tensor.matmul(
                            hp,
                            dg[:, j * N + n, :],
                            s_mm(c, j, n),
                            start=(n == 0),
                            stop=(n == N - 1),
                        )
                    hps.append(hp)
                st["hps"] = hps

            def emit_copies(c):
                """PSUM -> SBUF (+ bf16 cast) + stores; j=0 ACT, j=1 DVE."""
                st = state[c]
                hs = out_pool.tile([P, JT, D], out_dt, tag="hs")
                for j in range(JT):
                    hp = st["hps"][j]
                    # j1 always DVE; j0 mostly ACT, every 3rd macro DVE --
                    # measured balance point (ACT ~7us busier otherwise).
                    if j == 0 and c % 3 != 0:
                        nc.scalar.activation(out=hs[:, j, :], in_=hp, func=Act.Copy)
                    else:
                        nc.vector.tensor_copy(out=hs[:, j, :], in_=hp)
                    # SWDGE store from the idle GpSimd queue: a store that
                    # waits on its PSUM copy must not block next macros'
                    # loads in the (in-order) Sync HWDGE queue.
                    nc.gpsimd.dma_start(out=out_t[c, j], in_=hs[:, j, :])
                del state[c]

            # Software-pipelined emission. Per iteration i the engine queues
            # see (in order):
            #   DVE: sume/recip/diag(i-1), cast(i-2)... score/nmx(i), dots(i+1)
            #   ACT: squares(i+1)+Ln/Exp(i+1), exp(i), copy(i-1)
            #   PE : matmuls(i-1)
            # so every cross-engine wait lands behind a long runway of
            # already-ready work.
            emit_w()
            emit_loads(0)
            emit_loads(1)
            emit_idn()
            emit_passes(0)
            for c in range(nmacro):
                if c >= 1:
                    emit_back(c - 1)
                emit_front(c)
                if c + 2 < nmacro:
                    emit_loads(c + 2)
                if c + 1 < nmacro:
                    emit_passes(c + 1)
                emit_exp(c)
                if c >= 1:
                    emit_copies(c - 1)
            emit_back(nmacro - 1)
            emit_copies(nmacro - 1)

    nc.compile()
    return nc


def _get_nc(t_len=T, **kw):
    key = (t_len, tuple(sorted(kw.items())))
    if key not in _CACHE:
        _CACHE[key] = _build_bass(t_len, **kw)
    return _CACHE[key]


def _make_in_maps(sources, queries, layer_idx):
    sources = np.ascontiguousarray(np.asarray(sources, dtype=np.float32))
    queries = np.asarray(queries, dtype=np.float32)
    w = queries[int(layer_idx)]
    w_rep = np.ascontiguousarray(np.broadcast_to(w[None, :], (P, D)).astype(np.float32))
    idn = np.eye(P, dtype=np.float32)
    return [
        {"src": np.ascontiguousarray(sources[b]), "wq": w_rep, "idn": idn}
        for b in range(sources.shape[0])
    ]


def kernel(sources, queries, layer_idx):
    from concourse.bass_utils import run_bass_kernel_spmd

    nc = _get_nc()
    in_maps = _make_in_maps(sources, queries, layer_idx)
    res = run_bass_kernel_spmd(nc, in_maps, core_ids=list(range(NCORES)))
    outs = [
        np.asarray(res.results[b]["out"]).astype(np.float32) for b in range(NCORES)
    ]
    return np.stack(outs, axis=0)


# revision 28
# speedup vs baseline: 1.0244x; 1.0049x over previous
"""Trainium2 Bass kernel: BlockAttnRes forward (v2).

Reference computation (per batch b, position t):
    k[n]   = s[n] / sqrt(mean(s[n]^2) + eps)        n in [0, 9)
    score  = k[n] . w                                (w = queries[layer_idx])
    alpha  = softmax(score over n)
    h[t]   = sum_n alpha[n] * s[n]                   (d = 512)

Distribution: batch dim B=8 -> one batch per NeuronCore, no cross-core
communication.  Per core: T=4096 positions in 16 MACRO-tiles of 2x128
(two partition-tiles j=0,1 per macro -> small ops amortize over both).

Engine budget per macro (2 tiles, ~16us each on DVE/ACT = the wall):
    DMA   : 6 chunk loads [128,3,512] f32 (6KB rows) + 2 stores bf16
    ACT   : 18x Square+accum (ssq), Ln+Exp (rsq), 1x Exp (e), 1 PSUM copy
    DVE   : 18x dot STT+accum, score mul, seg max/sum, max-subtract via
            stride-0 broadcast add, recip, 2x diag STT (I*rs_j*e_j
            broadcast), 1 PSUM copy (bf16 cast)
    PE    : 18 accumulating fp32r matmuls (bitcast; 1 cyc/row vs 4 for f32)
Emission is software-pipelined across 3 macros so each engine's in-order
queue always holds ready work ahead of cross-engine waits. GpSimd is left
idle on purpose: it shares a DVE SBUF port and measurably slows the dot
STTs (tested: diag on GpSimd = +35us wall).

All ACT funcs (Square/Ln/Exp/Copy) in `natural_log_exp_and_others` ->
one ACT_TABLE_LOAD (pinned via PinnedBacc; stock chooser thrashes sets).
"""

import numpy as np

B, T, N, D = 8, 4096, 9, 512
P = 128
EPS = 1e-6
NCORES = 8
JT = 2              # partition-tiles per macro iteration
MACRO = P * JT      # 256 positions per macro

_CACHE = {}


def _build_bass(
    t_len=T,
    diag_engine="vector",   # "gpsimd" | "vector"
    n_act_sq=18,            # squares on ACT (rest on DVE via STT)
    out_bf16=True,
):
    import concourse.bass as bass
    import concourse.tile as tile
    from concourse import bacc, mybir

    f32 = mybir.dt.float32
    f32r = mybir.dt.float32r
    bf16 = mybir.dt.bfloat16
    Alu = mybir.AluOpType
    Act = mybir.ActivationFunctionType
    Ax = mybir.AxisListType

    nmacro = t_len // MACRO
    out_dt = bf16 if out_bf16 else f32

    PINNED_SET = "natural_log_exp_and_others"

    class PinnedBacc(bacc.Bacc):
        def insert_act_table_loads(self):
            import bass_rust as _bass_rust
            from concourse.hw_specs import get_activation_tables

            all_tables = get_activation_tables(self.m.arch)
            used = {
                i.func
                for b in self.main_func.blocks
                for i in b.instructions
                if isinstance(i, mybir.InstActivation)
            }
            if used and PINNED_SET in all_tables and used <= all_tables[PINNED_SET]:
                tables = [
                    (name, funcs if name == PINNED_SET else set())
                    for name, funcs in all_tables.items()
                ]
            else:
                tables = list(all_tables.items())
            _bass_rust.insert_act_table_loads(self, tables)

    nc = PinnedBacc("TRN2", target_bir_lowering=False, debug=False)
    src = nc.dram_tensor("src", [t_len, N, D], f32, kind="ExternalInput").ap()
    wq = nc.dram_tensor("wq", [P, D], f32, kind="ExternalInput").ap()
    idn = nc.dram_tensor("idn", [P, P], f32, kind="ExternalInput").ap()
    out = nc.dram_tensor("out", [t_len, D], out_dt, kind="ExternalOutput").ap()

    src_t = src.rearrange("(c j p) n d -> c j p n d", j=JT, p=P)
    out_t = out.rearrange("(c j p) d -> c j p d", j=JT, p=P)

    NCH = 3           # n-chunks per partition-tile
    CN = N // NCH     # n's per chunk

    def bc(ap, reps):
        """Insert a stride-0 dim after the partition dim."""
        return bass.AP(
            tensor=ap.tensor,
            offset=ap.offset,
            ap=[ap.ap[0], [0, reps], *ap.ap[1:]],
        )

    def bc_inner(ap, reps):
        """Append a stride-0 innermost dim."""
        return bass.AP(
            tensor=ap.tensor,
            offset=ap.offset,
            ap=[*ap.ap, [0, reps]],
        )

    with tile.TileContext(nc) as tc:
        with (
            tc.tile_pool(name="const", bufs=1) as const_pool,
            tc.tile_pool(name="srcp", bufs=4) as src_pool,
            tc.tile_pool(name="scratch", bufs=3) as scr_pool,
            tc.tile_pool(name="small", bufs=6) as small_pool,
            tc.tile_pool(name="diag", bufs=3) as diag_pool,
            tc.tile_pool(name="hout", bufs=6) as out_pool,
            tc.tile_pool(name="psum", bufs=4, space="PSUM") as psum_pool,
        ):
            state = {}
            const_tiles = {}

            def emit_w():
                # w gates the first dot STTs -> its DMA goes first of all.
                w_sb = const_pool.tile([P, D], f32, name="w_sb")
                nc.sync.dma_start(out=w_sb, in_=wq)
                eps_sb = const_pool.tile([P, 1], f32, name="eps_sb")
                nc.vector.memset(eps_sb, EPS)
                const_tiles.update(w=w_sb, eps=eps_sb)

            def emit_idn():
                # identity is first needed by diag in emit_back(0), much later
                i_sb = const_pool.tile([P, P], f32, name="i_sb")
                nc.sync.dma_start(out=i_sb, in_=idn)
                const_tiles.update(i=i_sb)

            def emit_loads(c):
                # Tiles are float32r-typed (verifier demands fp32r matmul
                # inputs come from fp32r locations); DVE/ACT consumers read
                # them bitcast back to f32 — same bytes.
                chunks = [[None] * NCH for _ in range(JT)]
                for j in range(JT):
                    for k in range(NCH):
                        sk = src_pool.tile([P, CN, D], f32r, tag=f"s{j}{k}")
                        nc.sync.dma_start(
                            out=sk,
                            in_=src_t[c, j, :, k * CN : (k + 1) * CN, :].bitcast(f32r),
                        )
                        chunks[j][k] = sk
                state[c] = {"chunks": chunks}

            def s_mm(c, j, n):
                return state[c]["chunks"][j][n // CN][:, n % CN, :]

            def s_of(c, j, n):
                return s_mm(c, j, n).bitcast(f32)

            def emit_passes(c):
                """Bulk streaming passes: ssq (ACT), dots (DVE), rsq (ACT)."""
                st = state[c]
                ssq = small_pool.tile([P, JT, N], f32, tag="ssq")
                sq = scr_pool.tile([P, D], f32, tag="sq")
                for j in range(JT):
                    for n in range(N):
                        nc.scalar.activation(
                            out=sq,
                            in_=s_of(c, j, n),
                            func=Act.Square,
                            accum_out=ssq[:, j, n : n + 1],
                        )
                dot = small_pool.tile([P, JT, N], f32, tag="dot")
                prod = scr_pool.tile([P, D], f32, tag="prod")
                for j in range(JT):
                    for n in range(N):
                        nc.vector.scalar_tensor_tensor(
                            out=prod,
                            in0=s_of(c, j, n),
                            scalar=0.0,
                            in1=const_tiles["w"],
                            op0=Alu.bypass,
                            op1=Alu.mult,
                            accum_out=dot[:, j, n : n + 1],
                        )
                # rsq = (ssq/D + eps)^(-1/2) via Exp(-0.5*Ln(x))
                rsq = small_pool.tile([P, JT, N], f32, tag="rsq")
                nc.scalar.activation(
                    out=rsq,
                    in_=ssq,
                    func=Act.Ln,
                    scale=1.0 / D,
                    bias=const_tiles["eps"],
                )
                nc.scalar.activation(out=rsq, in_=rsq, func=Act.Exp, scale=-0.5)
                st["dot"], st["rsq"] = dot, rsq

            def emit_front(c):
                """score + (negated) row max on DVE."""
                st = state[c]
                score = small_pool.tile([P, JT, N], f32, tag="score")
                nc.vector.tensor_mul(score, st["dot"], st["rsq"])
                nmx = small_pool.tile([P, JT], f32, tag="nmx")
                nc.vector.tensor_reduce(
                    out=nmx, in_=score, axis=Ax.X, op=Alu.max, negate=True
                )
                st["score"], st["nmx"] = score, nmx

            def emit_exp(c):
                """e = exp(score - max): subtract the per-j max on DVE via
                a stride-0 broadcast add (nmx is stored negated), then ONE
                ACT Exp over both tiles. Emitted late so the DVE inputs are
                long done when ACT reaches the Exp."""
                st = state[c]
                score2 = small_pool.tile([P, JT, N], f32, tag="score2")
                nc.vector.tensor_add(score2, st["score"], bc_inner(st["nmx"], N))
                e = small_pool.tile([P, JT, N], f32, tag="e")
                nc.scalar.activation(out=e, in_=score2, func=Act.Exp)
                st["e"] = e

            def emit_back(c):
                """sume/recip/diag on DVE + the 18 fp32r matmuls."""
                st = state[c]
                e = st["e"]
                sume = small_pool.tile([P, JT], f32, tag="sume")
                nc.vector.tensor_reduce(out=sume, in_=e, axis=Ax.X, op=Alu.add)
                rs = small_pool.tile([P, JT], f32, tag="rs")
                nc.vector.reciprocal(out=rs, in_=sume)

                # diag(alpha): either one GpSimd tensor mul over a
                # pre-built alpha (Pool has no scalar-AP ops), or per-j
                # DVE STTs folding rs — interleaved with that j's matmuls.
                dg = diag_pool.tile([P, JT * N, P], f32r, tag="dg")
                if diag_engine == "gpsimd":
                    al = small_pool.tile([P, JT * N], f32, tag="al")
                    for j in range(JT):
                        nc.vector.tensor_scalar_mul(
                            al[:, j * N : (j + 1) * N], e[:, j, :], rs[:, j : j + 1]
                        )
                    nc.gpsimd.tensor_tensor(
                        out=dg,
                        in0=bc(const_tiles["i"], JT * N),
                        in1=bc_inner(al, P),
                        op=Alu.mult,
                    )
                hps = []
                for j in range(JT):
                    if diag_engine != "gpsimd":
                        nc.vector.scalar_tensor_tensor(
                            out=dg[:, j * N : (j + 1) * N, :],
                            in0=bc(const_tiles["i"], N),
                            scalar=rs[:, j : j + 1],
                            in1=bc_inner(e[:, j, :], P),
                            op0=Alu.mult,
                            op1=Alu.mult,
                        )
                    hp = psum_pool.tile([P, D], f32, tag=f"hp{j}")
                    for n in range(N):
                        nc.tensor.matmul(
                            hp,
                            dg[:, j * N + n, :],
                            s_mm(c, j, n),
                            start=(n == 0),
                            stop=(n == N - 1),
                        )
                    hps.append(hp)
                st["hps"] = hps

            def emit_copies(c):
                """PSUM -> SBUF (+ bf16 cast) + stores; j=0 ACT, j=1 DVE."""
                st = state[c]
                hs = out_pool.tile([P, JT, D], out_dt, tag="hs")
                last = c == nmacro - 1
                for j in range(JT):
                    hp = st["hps"][j]
                    # j1 always DVE; j0 mostly ACT, every 3rd macro DVE --
                    # measured balance point (ACT ~7us busier otherwise).
                    # Last macro: both copies on ACT (idle in the drain
                    # while DVE runs the diag) and stores on the HWDGE
                    # queue (empty by then; SWDGE adds ~1us tail latency).
                    if last or (j == 0 and c % 3 != 0):
                        nc.scalar.activation(out=hs[:, j, :], in_=hp, func=Act.Copy)
                    else:
                        nc.vector.tensor_copy(out=hs[:, j, :], in_=hp)
                    if last:
                        nc.sync.dma_start(out=out_t[c, j], in_=hs[:, j, :])
                    else:
                        # SWDGE store from the idle GpSimd queue: a store
                        # that waits on its PSUM copy must not block next
                        # macros' loads in the in-order Sync HWDGE queue.
                        nc.gpsimd.dma_start(out=out_t[c, j], in_=hs[:, j, :])
                del state[c]

            # Software-pipelined emission. Per iteration i the engine queues
            # see (in order):
            #   DVE: sume/recip/diag(i-1), cast(i-2)... score/nmx(i), dots(i+1)
            #   ACT: squares(i+1)+Ln/Exp(i+1), exp(i), copy(i-1)
            #   PE : matmuls(i-1)
            # so every cross-engine wait lands behind a long runway of
            # already-ready work.
            emit_w()
            emit_loads(0)
            emit_loads(1)
            emit_idn()
            emit_passes(0)
            for c in range(nmacro):
                if c >= 1:
                    emit_back(c - 1)
                emit_front(c)
                if c + 2 < nmacro:
                    emit_loads(c + 2)
                if c + 1 < nmacro:
                    emit_passes(c + 1)
                emit_exp(c)
                if c >= 1:
                    emit_copies(c - 1)
            emit_back(nmacro - 1)
            emit_copies(nmacro - 1)

    nc.compile()
    return nc


def _get_nc(t_len=T, **kw):
    key = (t_len, tuple(sorted(kw.items())))
    if key not in _CACHE:
        _CACHE[key] = _build_bass(t_len, **kw)
    return _CACHE[key]


def _make_in_maps(sources, queries, layer_idx):
    sources = np.ascontiguousarray(np.asarray(sources, dtype=np.float32))
    queries = np.asarray(queries, dtype=np.float32)
    w = queries[int(layer_idx)]
    w_rep = np.ascontiguousarray(np.broadcast_to(w[None, :], (P, D)).astype(np.float32))
    idn = np.eye(P, dtype=np.float32)
    return [
        {"src": np.ascontiguousarray(sources[b]), "wq": w_rep, "idn": idn}
        for b in range(sources.shape[0])
    ]


def kernel(sources, queries, layer_idx):
    from concourse.bass_utils import run_bass_kernel_spmd

    nc = _get_nc()
    in_maps = _make_in_maps(sources, queries, layer_idx)
    res = run_bass_kernel_spmd(nc, in_maps, core_ids=list(range(NCORES)))
    outs = [
        np.asarray(res.results[b]["out"]).astype(np.float32) for b in range(NCORES)
    ]
    return np.stack(outs, axis=0)
